# revision 17
# baseline (speedup 1.0000x reference)
"""AMP loss kernel for Trainium2, 8 NeuronCores, pure data parallel.

Computes the reference AMP loss (masked SMAPE category sums + mean/var
costs + scale cost) over N=4,194,304 samples.  Each core processes a
contiguous 1/8 shard laid out as (128 partitions, 4096 free).  All big
reductions are done as PSUM-accumulated TensorE matmuls; elementwise work
runs in bf16 on DVE/ACT.  Host combines 8 cores' partial stats in fp64.
"""

import math

import numpy as np

import concourse.mybir as mybir
from concourse import bacc, bass
from concourse.tile import TileContext
from concourse.bass_utils import run_bass_kernel_spmd

N = 4_194_304
NCORES = 8
PER = N // NCORES          # 524288 elements per core
P = 128                    # partitions
F = PER // P               # 4096 free per partition
FC = 1024                  # compute chunk (free cols per chunk)
NCHUNK = F // FC           # 4
SUBS = FC // 128           # PE sub-chunks per chunk (8)
SHIFT = -120.0             # center all arrays near 0 for bf16/variance
LN2 = math.log(2.0)

F32 = mybir.dt.float32
BF16 = mybir.dt.bfloat16
ALU = mybir.AluOpType
AF = mybir.ActivationFunctionType

# out columns: 5 masked psums (256 each) | plain (384) | squares (512)
#              | counts (8) | moment sums (16)
COL_MASK = 0           # 5 * 256 = 1280
COL_PLAIN = 1280       # 384
COL_SQ = 1664          # 512
COL_CNT = 2176         # 8
COL_STATS = 2184       # 16
OUT_W = 2200

_CACHE = {}


def _build_nc():
    nc = bacc.Bacc(None, target_bir_lowering=False)
    d_dp = nc.declare_dram_parameter("dbp_pred", [P, F], F32, isOutput=False)
    d_sp = nc.declare_dram_parameter("sbp_pred", [P, F], F32, isOutput=False)
    d_d = nc.declare_dram_parameter("d", [P, F], F32, isOutput=False)
    d_s = nc.declare_dram_parameter("s", [P, F], F32, isOutput=False)
    d_out = nc.declare_dram_parameter("out", [P, OUT_W], F32, isOutput=True)

    with TileContext(nc) as tc:
        with (
            tc.tile_pool(name="const", bufs=1) as const_pool,
            tc.tile_pool(name="psum", bufs=1, space="PSUM") as psum_pool,
            tc.tile_pool(name="stats", bufs=1) as stats_pool,
            tc.tile_pool(name="inp", bufs=4) as inp_pool,
            tc.tile_pool(name="mom", bufs=2) as mom_pool,
            tc.tile_pool(name="pred", bufs=1) as pred_pool,
            tc.tile_pool(name="mask", bufs=2) as mask_pool,
            tc.tile_pool(name="smo", bufs=2) as smo_pool,
            tc.tile_pool(name="tmp", bufs=1) as tmp_pool,
            tc.tile_pool(name="inv", bufs=2) as inv_pool,
            tc.tile_pool(name="stage", bufs=1) as stage_pool,
        ):
            ones = const_pool.tile([P, 128], BF16)
            nc.vector.memset(ones[:], 1.0)
            bias_den = const_pool.tile([P, 1], F32)
            nc.vector.memset(bias_den[:], -2.0 * SHIFT)
            bias_ln2 = const_pool.tile([P, 1], F32)
            nc.vector.memset(bias_ln2[:], LN2)

            # persistent accumulators
            p_m = [psum_pool.tile([P, 256], F32, tag=f"pm{i}", name=f"pm{i}") for i in range(5)]
            p_plain = psum_pool.tile([P, 384], F32, tag="pplain")
            p_sq = psum_pool.tile([P, 512], F32, tag="psq")
            p_cnt = psum_pool.tile([P, 8], F32, tag="pcnt")
            stats = stats_pool.tile([P, 16], F32)
            nc.vector.memset(stats[:], 0.0)

            for c in range(NCHUNK):
                cs = bass.ts(c, FC)
                tin = {}
                for nm, dram in (("dp", d_dp), ("sp", d_sp), ("d", d_d), ("s", d_s)):
                    t = inp_pool.tile([P, FC], F32, tag=f"in_{nm}", name=f"in_{nm}")
                    nc.gpsimd.dma_start(out=t[:], in_=dram[:, cs])
                    tin[nm] = t

                # casts with shift; accum gives per-chunk sum(x')
                mom = mom_pool.tile([P, 4, FC], BF16)
                for j, nm in enumerate(("d", "s", "dp", "sp")):
                    nc.scalar.activation(
                        mom[:, j, :], tin[nm][:], AF.Copy, bias=SHIFT,
                        accum_out=stats[:, j * NCHUNK + c : j * NCHUNK + c + 1],
                    )
                dT, sT, dpT, spT = (mom[:, j, :] for j in range(4))

                # predicates (shifted thresholds)
                pr = pred_pool.tile([P, 7, FC], BF16)
                a1, a2, b1, c1, d1, e1, e2 = (pr[:, k, :] for k in range(7))
                nc.vector.tensor_scalar(a1, sT, 0.0, None, ALU.is_lt)     # s < 120
                nc.vector.tensor_scalar(a2, dT, -40.0, None, ALU.is_lt)   # d < 80
                nc.vector.tensor_scalar(b1, sT, 10.0, None, ALU.is_lt)    # s < 130
                nc.vector.tensor_scalar(c1, sT, 20.0, None, ALU.is_lt)    # s < 140
                nc.vector.tensor_scalar(d1, dT, -30.0, None, ALU.is_lt)   # d < 90
                nc.vector.tensor_scalar(e1, sT, 60.0, None, ALU.is_gt)    # s > 180
                nc.vector.tensor_scalar(e2, dT, 0.0, None, ALU.is_gt)     # d > 120

                # masks: m0, r=b1*a2, q=c1*d1, m2, m4
                mk = mask_pool.tile([P, 5, FC], BF16)
                nc.vector.tensor_tensor(mk[:, 0, :], a1, a2, ALU.mult)
                nc.vector.tensor_tensor(mk[:, 1, :], b1, a2, ALU.mult)
                nc.vector.tensor_tensor(mk[:, 2, :], c1, d1, ALU.mult)
                u = tmp_pool.tile([P, FC], BF16, tag="u")
                v = tmp_pool.tile([P, FC], BF16, tag="v")
                nc.vector.tensor_tensor(u[:], c1, b1, ALU.subtract)
                nc.vector.tensor_tensor(v[:], d1, a2, ALU.subtract)
                nc.vector.tensor_tensor(mk[:, 3, :], u[:], v[:], ALU.max)
                nc.vector.tensor_tensor(mk[:, 4, :], e1, e2, ALU.max)

                # smo = [smd, sms, lt]
                smo = smo_pool.tile([P, 3, FC], BF16)
                nc.vector.tensor_tensor(smo[:, 2, :], dpT, spT, ALU.is_lt)
                for side, (pp, tt) in enumerate(((dpT, dT), (spT, sT))):
                    diff = tmp_pool.tile([P, FC], BF16, tag=f"diff{side}", name=f"diff{side}")
                    adiff = tmp_pool.tile([P, FC], BF16, tag=f"adiff{side}", name=f"adiff{side}")
                    den = tmp_pool.tile([P, FC], BF16, tag=f"den{side}", name=f"den{side}")
                    lnt = tmp_pool.tile([P, FC], F32, tag=f"ln{side}", name=f"ln{side}")
                    inv = inv_pool.tile([P, FC], BF16, tag=f"inv{side}", name=f"inv{side}")
                    nc.vector.tensor_tensor(diff[:], pp, tt, ALU.subtract)
                    nc.scalar.activation(adiff[:], diff[:], AF.Abs)
                    nc.vector.tensor_tensor(den[:], pp, tt, ALU.add)
                    nc.scalar.activation(lnt[:], den[:], AF.Ln, bias=bias_den[:])
                    nc.scalar.activation(inv[:], lnt[:], AF.Exp, bias=bias_ln2[:], scale=-1.0)
                    nc.vector.tensor_tensor(smo[:, side, :], adiff[:], inv[:], ALU.mult)

                # PE reductions per 128-col sub-chunk
                for sgrp in range(SUBS):
                    ss = bass.ts(sgrp, 128)
                    first = c == 0 and sgrp == 0
                    last = c == NCHUNK - 1 and sgrp == SUBS - 1
                    kw = dict(start=first, stop=last)
                    nc.tensor.matmul(p_plain[:], ones[:], smo[:, :, ss], **kw)
                    for i in range(5):
                        nc.tensor.matmul(p_m[i][:], mk[:, i, ss], smo[:, 0:2, ss], **kw)
                        # p_cnt shares one PSUM bank across all 5 column
                        # groups: start may only be set on the bank's very
                        # first matmul (start clears has_written bank-wide).
                        nc.tensor.matmul(
                            p_cnt[:, i : i + 1], mk[:, i, ss], ones[:, 0:1],
                            start=first and i == 0, stop=last and i == 4,
                            skip_group_check=True,
                        )
                    for j in range(4):
                        nc.tensor.matmul(
                            p_sq[:, bass.ts(j, 128)], mom[:, j, ss], mom[:, j, ss],
                            start=first and j == 0, stop=last and j == 3,
                            skip_group_check=True,
                        )

            # evacuate psums -> sbuf -> dram
            stage = stage_pool.tile([P, COL_STATS], F32)
            for i in range(5):
                nc.scalar.copy(stage[:, bass.ts(i, 256)], p_m[i][:])
            nc.scalar.copy(stage[:, COL_PLAIN : COL_PLAIN + 384], p_plain[:])
            nc.scalar.copy(stage[:, COL_SQ : COL_SQ + 512], p_sq[:])
            nc.scalar.copy(stage[:, COL_CNT : COL_CNT + 8], p_cnt[:])
            nc.scalar.dma_start(out=d_out[:, 0:COL_STATS], in_=stage[:])
            nc.scalar.dma_start(out=d_out[:, COL_STATS:OUT_W], in_=stats[:])

    nc.finalize()
    return nc


def _get_nc():
    if "nc" not in _CACHE:
        _CACHE["nc"] = _build_nc()
    return _CACHE["nc"]


def _shard(x):
    x = np.ascontiguousarray(np.asarray(x, dtype=np.float32).reshape(-1))
    return [x[i * PER : (i + 1) * PER].reshape(P, F) for i in range(NCORES)]


def _host_finish(outs):
    """outs: list of 8 (P, OUT_W) float32 arrays -> loss tuple."""
    diag = np.arange(128)
    msum_d = np.zeros(5)
    msum_s = np.zeros(5)
    cnt_pe = np.zeros(5)
    s_smd = s_sms = n_lt = 0.0
    sq = np.zeros(4)
    s1 = np.zeros(4)
    for O in outs:
        O = O.astype(np.float64)
        for i in range(5):
            blk = O[:, i * 256 : (i + 1) * 256]
            msum_d[i] += blk[diag, diag].sum()
            msum_s[i] += blk[diag, 128 + diag].sum()
        plain = O[:, COL_PLAIN : COL_PLAIN + 384]
        s_smd += plain[0, 0:128].sum()
        s_sms += plain[0, 128:256].sum()
        n_lt += plain[0, 256:384].sum()
        sqb = O[:, COL_SQ : COL_SQ + 512]
        for j in range(4):
            sq[j] += sqb[diag, j * 128 + diag].sum()
        cnt_pe += O[:, COL_CNT : COL_CNT + 5].sum(axis=0)
        st = O[:, COL_STATS:OUT_W]
        for j in range(4):
            s1[j] += st[:, j * NCHUNK : (j + 1) * NCHUNK].sum()

    # masks were [m0, r=m0+m1, q=~m3, m2, m4]
    counts = np.array(
        [cnt_pe[0], cnt_pe[1] - cnt_pe[0], cnt_pe[3], N - cnt_pe[2], cnt_pe[4]]
    )
    sum_d_raw = np.array(
        [msum_d[0], msum_d[1] - msum_d[0], msum_d[3], s_smd - msum_d[2], msum_d[4]]
    )
    sum_s_raw = np.array(
        [msum_s[0], msum_s[1] - msum_s[0], msum_s[3], s_sms - msum_s[2], msum_s[4]]
    )

    nf = float(N)
    safe = np.maximum(counts, 1.0)
    w = np.sqrt(np.log(nf / safe))
    sum_d = w * sum_d_raw
    sum_s = w * sum_s_raw
    has = counts > 0
    d_rst = s_rst = rst_d = rst_s = 0.0
    for i in range(5):
        nd = (d_rst + sum_d[i]) / safe[i]
        ns = (s_rst + sum_s[i]) / safe[i]
        if has[i]:
            d_rst, s_rst = nd, ns
            rst_d += nd
            rst_s += ns
    cnt = int(has.sum())
    denom = 5.0 if cnt == 0 else float(cnt)
    rst_d /= denom
    rst_s /= denom

    # order in mom/stats: d, s, dp, sp ; x' = x + SHIFT
    mean = -SHIFT + s1 / nf
    var = (sq - s1 * s1 / nf) / (nf - 1)
    dbp_mean_cost = abs(mean[0] - mean[2]) / mean[0]
    sbp_mean_cost = abs(mean[1] - mean[3]) / mean[1]
    dbp_var_cost = abs(var[0] - var[2]) / var[0]
    sbp_var_cost = abs(var[1] - var[3]) / var[1]
    scale_cost = 1.0 - n_lt / nf

    return (
        np.float32(rst_d + dbp_mean_cost + dbp_var_cost),
        np.float32(rst_s + sbp_mean_cost + sbp_var_cost),
        np.float32(scale_cost),
    )


def _get_runner():
    """Jitted 8-core NEFF runner (mirrors bass2jax.run_bass_via_pjrt),
    cached so repeated calls don't retrace/recompile."""
    if "runner" in _CACHE:
        return _CACHE["runner"]
    import jax
    from jax.sharding import Mesh, PartitionSpec
    from jax.experimental.shard_map import shard_map
    from concourse import bass2jax
    from concourse.bass2jax import _bass_exec_p, install_neuronx_cc_hook
    import concourse.mybir as _mybir

    nc = _get_nc()
    assert nc.dbg_addr is None
    part_name = nc.partition_id_tensor.name if nc.partition_id_tensor else None
    install_neuronx_cc_hook()
    in_names, out_names, out_avals, zero_outs = [], [], [], []
    for alloc in nc.m.functions[0].allocations:
        if not isinstance(alloc, _mybir.MemoryLocationSet):
            continue
        name = alloc.memorylocations[0].name
        if alloc.kind == "ExternalInput":
            if name != part_name:
                in_names.append(name)
        elif alloc.kind == "ExternalOutput":
            shape = tuple(alloc.tensor_shape)
            dtype = _mybir.dt.np(alloc.dtype)
            out_names.append(name)
            out_avals.append(jax.core.ShapedArray(shape, dtype))
            zero_outs.append(np.zeros(shape, dtype))
    n_params = len(in_names)
    all_in_names = in_names + out_names
    if part_name is not None:
        all_in_names = all_in_names + [part_name]

    def _body(*args):
        operands = list(args)
        if part_name is not None:
            operands.append(bass2jax.partition_id_tensor())
        outs = _bass_exec_p.bind(
            *operands,
            out_avals=tuple(out_avals),
            in_names=tuple(all_in_names),
            out_names=tuple(out_names),
            lowering_input_output_aliases=(),
            sim_require_finite=True,
            sim_require_nnan=True,
            nc=nc,
        )
        return tuple(outs)

    import jax as _jax

    devices = _jax.devices()[:NCORES]
    mesh = Mesh(np.asarray(devices), ("core",))
    n_outs = len(out_names)
    sharded = _jax.jit(
        shard_map(
            _body,
            mesh=mesh,
            in_specs=(PartitionSpec("core"),) * (n_params + n_outs),
            out_specs=(PartitionSpec("core"),) * n_outs,
            check_rep=False,
        ),
        donate_argnums=tuple(range(n_params, n_params + n_outs)),
        keep_unused=True,
    )
    _CACHE["runner"] = (sharded, in_names, out_names, zero_outs)
    return _CACHE["runner"]


def _run_device(in_maps):
    """Execute once on 8 cores; returns list of per-core out arrays."""
    import jax

    sharded, in_names, out_names, zero_outs = _get_runner()
    concat_in = [
        np.concatenate([in_maps[c][nm] for c in range(NCORES)], axis=0)
        for nm in in_names
    ]
    zeros = [np.concatenate([z] * NCORES, axis=0) for z in zero_outs]
    out = sharded(*concat_in, *zeros)
    out0 = np.asarray(out[0])
    per = out0.shape[0] // NCORES
    return [out0[i * per : (i + 1) * per] for i in range(NCORES)]


def kernel(dbp_pred, sbp_pred, mbp_pred, d, s, m, _bench=None):
    shards = {
        "dbp_pred": _shard(dbp_pred),
        "sbp_pred": _shard(sbp_pred),
        "d": _shard(d),
        "s": _shard(s),
    }
    in_maps = [{k: shards[k][i] for k in shards} for i in range(NCORES)]
    outs = _run_device(in_maps)
    _CACHE["last_outs"] = outs
    return _host_finish(outs)


# revision 21
# speedup vs baseline: 1.7657x; 1.7657x over previous
"""AMP loss kernel for Trainium2, 8 NeuronCores, pure data parallel.

Computes the reference AMP loss (masked SMAPE category sums + mean/var
costs + scale cost) over N=4,194,304 samples.  Each core processes a
contiguous 1/8 shard laid out as (128 partitions, 4096 free).  All big
reductions are done as PSUM-accumulated TensorE matmuls; elementwise work
runs in bf16 on DVE/ACT.  Host combines 8 cores' partial stats in fp64.
"""

import math

import numpy as np

import concourse.mybir as mybir
from concourse import bacc, bass
from concourse.tile import TileContext
from concourse.bass_utils import run_bass_kernel_spmd

N = 4_194_304
NCORES = 8
PER = N // NCORES          # 524288 elements per core
P = 128                    # partitions
F = PER // P               # 4096 free per partition
FC = 1024                  # compute chunk (free cols per chunk)
NCHUNK = F // FC           # 4
SUBS = FC // 128           # PE sub-chunks per chunk (8)
SHIFT = 0.0                # fp16 keeps enough mantissa unshifted
LN2 = math.log(2.0)

F32 = mybir.dt.float32
F16 = mybir.dt.float16
ALU = mybir.AluOpType
AF = mybir.ActivationFunctionType

# out columns: 5 masked psums (256 each) | plain (384) | squares (512)
#              | counts+moment sums (12: 5 counts, pad, 4 sums at 8..11)
COL_MASK = 0           # 5 * 256 = 1280
COL_PLAIN = 1280       # 384
COL_SQ = 1664          # 512
COL_CNT = 2176         # 12
OUT_W = 2188

_CACHE = {}


def _pin_act_table():
    """Make Copy/Abs/Ln/Exp all resolve to natural_log_exp_and_others so the
    kernel needs exactly one ACT table load (the insertion pass picks the
    first set containing each function, which otherwise thrashes between
    exp_and_others and natural_log every chunk, ~2.7us per load)."""
    from concourse.hw_specs import get_activation_tables

    tabs = get_activation_tables("gen3")  # functools.cache'd: mutate in place
    keep = {AF.Copy, AF.Abs, AF.Ln, AF.Exp}
    for name, funcs in tabs.items():
        if name != "natural_log_exp_and_others":
            funcs -= keep


def _build_nc():
    _pin_act_table()
    nc = bacc.Bacc(None, target_bir_lowering=False)
    d_dp = nc.declare_dram_parameter("dbp_pred", [P, F], F32, isOutput=False)
    d_sp = nc.declare_dram_parameter("sbp_pred", [P, F], F32, isOutput=False)
    d_d = nc.declare_dram_parameter("d", [P, F], F32, isOutput=False)
    d_s = nc.declare_dram_parameter("s", [P, F], F32, isOutput=False)
    d_out = nc.declare_dram_parameter("out", [P, OUT_W], F32, isOutput=True)

    with TileContext(nc) as tc:
        with (
            tc.tile_pool(name="const", bufs=1) as const_pool,
            tc.tile_pool(name="psum", bufs=1, space="PSUM") as psum_pool,
            tc.tile_pool(name="stats", bufs=1) as stats_pool,
            tc.tile_pool(name="mom", bufs=2) as mom_pool,
            tc.tile_pool(name="pred", bufs=1) as pred_pool,
            tc.tile_pool(name="mask", bufs=2) as mask_pool,
            tc.tile_pool(name="smo", bufs=2) as smo_pool,
            tc.tile_pool(name="tmp", bufs=1) as tmp_pool,
            tc.tile_pool(name="inv", bufs=2) as inv_pool,
            tc.tile_pool(name="stage", bufs=1) as stage_pool,
        ):
            ones = const_pool.tile([P, 128], F16)
            nc.vector.memset(ones[:], 1.0)
            bias_ln2 = const_pool.tile([P, 1], F32)
            nc.vector.memset(bias_ln2[:], LN2)

            # persistent accumulators
            p_m = [psum_pool.tile([P, 256], F32, tag=f"pm{i}", name=f"pm{i}") for i in range(5)]
            p_plain = psum_pool.tile([P, 384], F32, tag="pplain")
            p_sq = psum_pool.tile([P, 512], F32, tag="psq")
            p_cnt = psum_pool.tile([P, 12], F32, tag="pcnt")

            momt = None
            for c in range(NCHUNK):
                # casting DMA (fp32 dram -> fp16 sbuf); gpsimd-only feature.
                # Loaded two compute-chunks at a time to halve the Q7
                # descriptor-generation work.
                if c % 2 == 0:
                    momt = mom_pool.tile([P, 4, 2 * FC], F16, name="momt")
                    dcs = bass.ds(c * FC, 2 * FC)
                    for j, dram in enumerate((d_d, d_s, d_dp, d_sp)):
                        nc.gpsimd.dma_start(out=momt[:, j, :], in_=dram[:, dcs])
                off = (c % 2) * FC
                mom = momt[:, :, off : off + FC]
                dT, sT, dpT, spT = (mom[:, j, :] for j in range(4))

                # smape first: the ACT chain (abs -> ln -> exp) has the
                # longest latency and gates the PE's rhs, so start it
                # before the predicate/mask work.
                smo = smo_pool.tile([P, 3, FC], F16)
                sm_in = []
                for side, (pp, tt) in enumerate(((dpT, dT), (spT, sT))):
                    diff = tmp_pool.tile([P, FC], F16, tag=f"diff{side}", name=f"diff{side}", bufs=2)
                    adiff = tmp_pool.tile([P, FC], F16, tag=f"adiff{side}", name=f"adiff{side}", bufs=2)
                    den = tmp_pool.tile([P, FC], F16, tag=f"den{side}", name=f"den{side}", bufs=2)
                    lnt = tmp_pool.tile([P, FC], F32, tag=f"ln{side}", name=f"ln{side}", bufs=2)
                    inv = inv_pool.tile([P, FC], F16, tag=f"inv{side}", name=f"inv{side}")
                    nc.vector.tensor_tensor(diff[:], pp, tt, ALU.subtract)
                    nc.scalar.activation(adiff[:], diff[:], AF.Abs)
                    nc.vector.tensor_tensor(den[:], pp, tt, ALU.add)
                    nc.scalar.activation(lnt[:], den[:], AF.Ln)
                    nc.scalar.activation(inv[:], lnt[:], AF.Exp, bias=bias_ln2[:], scale=-1.0)
                    sm_in.append((adiff, inv))

                # predicates
                pr = pred_pool.tile([P, 7, FC], F16)
                a1, a2, b1, c1, d1, e1, e2 = (pr[:, k, :] for k in range(7))
                nc.vector.tensor_scalar(a1, sT, 120.0, None, ALU.is_lt)   # s < 120
                nc.vector.tensor_scalar(a2, dT, 80.0, None, ALU.is_lt)    # d < 80
                nc.vector.tensor_scalar(b1, sT, 130.0, None, ALU.is_lt)   # s < 130
                nc.vector.tensor_scalar(c1, sT, 140.0, None, ALU.is_lt)   # s < 140
                nc.vector.tensor_scalar(d1, dT, 90.0, None, ALU.is_lt)    # d < 90
                nc.vector.tensor_scalar(e1, sT, 180.0, None, ALU.is_gt)   # s > 180
                nc.vector.tensor_scalar(e2, dT, 120.0, None, ALU.is_gt)   # d > 120

                # masks: m0, r=b1*a2, q=c1*d1, m2, m4
                mk = mask_pool.tile([P, 5, FC], F16)
                nc.vector.tensor_tensor(mk[:, 0, :], a1, a2, ALU.mult)
                nc.vector.tensor_tensor(mk[:, 1, :], b1, a2, ALU.mult)
                nc.vector.tensor_tensor(mk[:, 2, :], c1, d1, ALU.mult)
                u = tmp_pool.tile([P, FC], F16, tag="u")
                v = tmp_pool.tile([P, FC], F16, tag="v")
                nc.vector.tensor_tensor(u[:], c1, b1, ALU.subtract)
                nc.vector.tensor_tensor(v[:], d1, a2, ALU.subtract)
                nc.vector.tensor_tensor(mk[:, 3, :], u[:], v[:], ALU.max)
                nc.vector.tensor_tensor(mk[:, 4, :], e1, e2, ALU.max)
                nc.vector.tensor_tensor(smo[:, 2, :], dpT, spT, ALU.is_lt)

                # smape products land in smo as soon as exp is done
                for side, (adiff, inv) in enumerate(sm_in):
                    nc.vector.tensor_tensor(smo[:, side, :], adiff[:], inv[:], ALU.mult)

                # PE reductions per 128-col sub-chunk
                for sgrp in range(SUBS):
                    ss = bass.ts(sgrp, 128)
                    first = c == 0 and sgrp == 0
                    last = c == NCHUNK - 1 and sgrp == SUBS - 1
                    kw = dict(start=first, stop=last)
                    nc.tensor.matmul(p_plain[:], ones[:], smo[:, :, ss], **kw)
                    for i in range(5):
                        nc.tensor.matmul(p_m[i][:], mk[:, i, ss], smo[:, 0:2, ss], **kw)
                        # p_cnt shares one PSUM bank across all 5 column
                        # groups: start may only be set on the bank's very
                        # first matmul (start clears has_written bank-wide).
                        nc.tensor.matmul(
                            p_cnt[:, i : i + 1], mk[:, i, ss], ones[:, 0:1],
                            start=first and i == 0, stop=False,
                            skip_group_check=True,
                        )
                    for j in range(4):
                        nc.tensor.matmul(
                            p_sq[:, bass.ts(j, 128)], mom[:, j, ss], mom[:, j, ss],
                            start=first and j == 0, stop=last and j == 3,
                            skip_group_check=True,
                        )
                        nc.tensor.matmul(
                            p_cnt[:, 8 + j : 9 + j], mom[:, j, ss], ones[:, 0:1],
                            start=False, stop=last and j == 3,
                            skip_group_check=True,
                        )

            # evacuate psums -> sbuf -> dram
            stage = stage_pool.tile([P, OUT_W], F32)
            for i in range(5):
                nc.scalar.copy(stage[:, bass.ts(i, 256)], p_m[i][:])
            nc.scalar.copy(stage[:, COL_PLAIN : COL_PLAIN + 384], p_plain[:])
            nc.scalar.copy(stage[:, COL_SQ : COL_SQ + 512], p_sq[:])
            nc.scalar.copy(stage[:, COL_CNT : COL_CNT + 12], p_cnt[:])
            nc.scalar.dma_start(out=d_out[:], in_=stage[:])

    nc.finalize()
    return nc


def _get_nc():
    if "nc" not in _CACHE:
        _CACHE["nc"] = _build_nc()
    return _CACHE["nc"]


def _shard(x):
    x = np.ascontiguousarray(np.asarray(x, dtype=np.float32).reshape(-1))
    return [x[i * PER : (i + 1) * PER].reshape(P, F) for i in range(NCORES)]


def _host_finish(outs):
    """outs: list of 8 (P, OUT_W) float32 arrays -> loss tuple."""
    diag = np.arange(128)
    msum_d = np.zeros(5)
    msum_s = np.zeros(5)
    cnt_pe = np.zeros(5)
    s_smd = s_sms = n_lt = 0.0
    sq = np.zeros(4)
    s1 = np.zeros(4)
    for O in outs:
        O = O.astype(np.float64)
        for i in range(5):
            blk = O[:, i * 256 : (i + 1) * 256]
            msum_d[i] += blk[diag, diag].sum()
            msum_s[i] += blk[diag, 128 + diag].sum()
        plain = O[:, COL_PLAIN : COL_PLAIN + 384]
        s_smd += plain[0, 0:128].sum()
        s_sms += plain[0, 128:256].sum()
        n_lt += plain[0, 256:384].sum()
        sqb = O[:, COL_SQ : COL_SQ + 512]
        for j in range(4):
            sq[j] += sqb[diag, j * 128 + diag].sum()
        cnt_pe += O[:, COL_CNT : COL_CNT + 5].sum(axis=0)
        for j in range(4):
            s1[j] += O[:, COL_CNT + 8 + j].sum()

    # masks were [m0, r=m0+m1, q=~m3, m2, m4]
    counts = np.array(
        [cnt_pe[0], cnt_pe[1] - cnt_pe[0], cnt_pe[3], N - cnt_pe[2], cnt_pe[4]]
    )
    sum_d_raw = np.array(
        [msum_d[0], msum_d[1] - msum_d[0], msum_d[3], s_smd - msum_d[2], msum_d[4]]
    )
    sum_s_raw = np.array(
        [msum_s[0], msum_s[1] - msum_s[0], msum_s[3], s_sms - msum_s[2], msum_s[4]]
    )

    nf = float(N)
    safe = np.maximum(counts, 1.0)
    w = np.sqrt(np.log(nf / safe))
    sum_d = w * sum_d_raw
    sum_s = w * sum_s_raw
    has = counts > 0
    d_rst = s_rst = rst_d = rst_s = 0.0
    for i in range(5):
        nd = (d_rst + sum_d[i]) / safe[i]
        ns = (s_rst + sum_s[i]) / safe[i]
        if has[i]:
            d_rst, s_rst = nd, ns
            rst_d += nd
            rst_s += ns
    cnt = int(has.sum())
    denom = 5.0 if cnt == 0 else float(cnt)
    rst_d /= denom
    rst_s /= denom

    # order in mom: d, s, dp, sp
    mean = s1 / nf
    var = (sq - s1 * s1 / nf) / (nf - 1)
    dbp_mean_cost = abs(mean[0] - mean[2]) / mean[0]
    sbp_mean_cost = abs(mean[1] - mean[3]) / mean[1]
    dbp_var_cost = abs(var[0] - var[2]) / var[0]
    sbp_var_cost = abs(var[1] - var[3]) / var[1]
    scale_cost = 1.0 - n_lt / nf

    return (
        np.float32(rst_d + dbp_mean_cost + dbp_var_cost),
        np.float32(rst_s + sbp_mean_cost + sbp_var_cost),
        np.float32(scale_cost),
    )


def _get_runner():
    """Jitted 8-core NEFF runner (mirrors bass2jax.run_bass_via_pjrt),
    cached so repeated calls don't retrace/recompile."""
    if "runner" in _CACHE:
        return _CACHE["runner"]
    import jax
    from jax.sharding import Mesh, PartitionSpec
    from jax.experimental.shard_map import shard_map
    from concourse import bass2jax
    from concourse.bass2jax import _bass_exec_p, install_neuronx_cc_hook
    import concourse.mybir as _mybir

    nc = _get_nc()
    assert nc.dbg_addr is None
    part_name = nc.partition_id_tensor.name if nc.partition_id_tensor else None
    install_neuronx_cc_hook()
    in_names, out_names, out_avals, zero_outs = [], [], [], []
    for alloc in nc.m.functions[0].allocations:
        if not isinstance(alloc, _mybir.MemoryLocationSet):
            continue
        name = alloc.memorylocations[0].name
        if alloc.kind == "ExternalInput":
            if name != part_name:
                in_names.append(name)
        elif alloc.kind == "ExternalOutput":
            shape = tuple(alloc.tensor_shape)
            dtype = _mybir.dt.np(alloc.dtype)
            out_names.append(name)
            out_avals.append(jax.core.ShapedArray(shape, dtype))
            zero_outs.append(np.zeros(shape, dtype))
    n_params = len(in_names)
    all_in_names = in_names + out_names
    if part_name is not None:
        all_in_names = all_in_names + [part_name]

    def _body(*args):
        operands = list(args)
        if part_name is not None:
            operands.append(bass2jax.partition_id_tensor())
        outs = _bass_exec_p.bind(
            *operands,
            out_avals=tuple(out_avals),
            in_names=tuple(all_in_names),
            out_names=tuple(out_names),
            lowering_input_output_aliases=(),
            sim_require_finite=True,
            sim_require_nnan=True,
            nc=nc,
        )
        return tuple(outs)

    import jax as _jax

    devices = _jax.devices()[:NCORES]
    mesh = Mesh(np.asarray(devices), ("core",))
    n_outs = len(out_names)
    sharded = _jax.jit(
        shard_map(
            _body,
            mesh=mesh,
            in_specs=(PartitionSpec("core"),) * (n_params + n_outs),
            out_specs=(PartitionSpec("core"),) * n_outs,
            check_rep=False,
        ),
        donate_argnums=tuple(range(n_params, n_params + n_outs)),
        keep_unused=True,
    )
    _CACHE["runner"] = (sharded, in_names, out_names, zero_outs)
    return _CACHE["runner"]


def _run_device(in_maps):
    """Execute once on 8 cores; returns list of per-core out arrays."""
    import jax

    sharded, in_names, out_names, zero_outs = _get_runner()
    concat_in = [
        np.concatenate([in_maps[c][nm] for c in range(NCORES)], axis=0)
        for nm in in_names
    ]
    zeros = [np.concatenate([z] * NCORES, axis=0) for z in zero_outs]
    out = sharded(*concat_in, *zeros)
    out0 = np.asarray(out[0])
    per = out0.shape[0] // NCORES
    return [out0[i * per : (i + 1) * per] for i in range(NCORES)]


def kernel(dbp_pred, sbp_pred, mbp_pred, d, s, m, _bench=None):
    shards = {
        "dbp_pred": _shard(dbp_pred),
        "sbp_pred": _shard(sbp_pred),
        "d": _shard(d),
        "s": _shard(s),
    }
    in_maps = [{k: shards[k][i] for k in shards} for i in range(NCORES)]
    outs = _run_device(in_maps)
    _CACHE["last_outs"] = outs
    return _host_finish(outs)
